# revision 12
# baseline (speedup 1.0000x reference)
"""Bahdanau attention Trainium2 kernel (v3: bf16 datapath).

Problem: B=64, T=2048, ENC=512, DEC=512, ATTN=256, fp32 in/out.
  proj_enc = enc_out @ W_enc                         [B,T,A]
  energy   = tanh(proj_enc + dec_hidden@W_dec) . v   [B,T]
  attn_w   = softmax(mask(energy))                   [B,T]
  context  = attn_w @ enc_out                        [B,E]

Sharding: data-parallel over batch across 8 cores (8 batches/core),
weights replicated.  Per core, batches are processed in pairs so the
softmax runs on [2, T] tiles (engine ops need 32-aligned base
partitions) while the pair's enc tiles stay resident in SBUF for the
context matmul (single HBM pass).

enc is cast fp32->bf16 during the DMA load (line rate).  All matmuls
run in bf16 with fp32 PSUM accumulation; softmax runs in fp32.  enc
tiles are PE-transposed (bf16, 1 cyc/row) for the projection matmul.
The context matmul uses 4-way column tiling (tile_position) with the
4 partial sums reduced on the host during unshard.
"""

import sys

sys.path.insert(0, "/opt/trn_rl_repo")

import numpy as np
import concourse.bass as bass
import concourse.tile as tile
from concourse import bacc, mybir
from concourse.bass_utils import run_bass_kernel_spmd
from concourse.masks import make_identity

dt = mybir.dt
F32 = dt.float32
BF16 = dt.bfloat16
U8 = dt.uint8
AF = mybir.ActivationFunctionType
ALU = mybir.AluOpType

B, T, ENC, DEC, ATTN = 64, 2048, 512, 512, 256
NCORES = 8
BL = B // NCORES  # 8 batches per core
GS = 4  # batches per group
NQ = BL // GS  # 2 quads
KC = ENC // 128  # 4 contraction chunks
MC = ATTN // 128  # 2 attn chunks
NG = T // 512  # 4 token groups of 512
NCH = T // 128  # 16 token chunks of 128
NEG = -1.0e30


def build_program(reps=1, timing_mode=False):
    nc = bacc.Bacc("TRN2", target_bir_lowering=False, debug=False)

    enc_kind = "Internal" if timing_mode else "ExternalInput"
    enc_d = nc.dram_tensor("enc_d", [BL, T, ENC], F32, kind=enc_kind).ap()
    dec_d = nc.dram_tensor("dec_d", [BL, DEC], F32, kind="ExternalInput").ap()
    msk_d = nc.dram_tensor("msk_d", [BL, T], U8, kind="ExternalInput").ap()
    we_d = nc.dram_tensor("we_d", [ENC, ATTN], F32, kind="ExternalInput").ap()
    wd_d = nc.dram_tensor("wd_d", [DEC, ATTN], F32, kind="ExternalInput").ap()
    v_d = nc.dram_tensor("v_d", [ATTN], F32, kind="ExternalInput").ap()
    ctx_d = nc.dram_tensor("ctx_d", [BL, 4, ENC], F32, kind="ExternalOutput").ap()
    att_d = nc.dram_tensor("att_d", [BL, T], F32, kind="ExternalOutput").ap()

    with tile.TileContext(nc) as tc:
        with (
            tc.tile_pool(name="consts", bufs=1) as cp,
            tc.tile_pool(name="nat0p", bufs=2) as nat0p,
            tc.tile_pool(name="nat1p", bufs=2) as nat1p,
            tc.tile_pool(name="nat2p", bufs=2) as nat2p,
            tc.tile_pool(name="nat3p", bufs=2) as nat3p,
            tc.tile_pool(name="encTp", bufs=6) as encTp,
            tc.tile_pool(name="thp", bufs=6) as thp,
            tc.tile_pool(name="erow", bufs=2) as erowp,
            tc.tile_pool(name="prow", bufs=2) as prowp,
            tc.tile_pool(name="smallp", bufs=4) as smallp,
            tc.tile_pool(name="wTp", bufs=2) as wTp,
            tc.tile_pool(name="ps_tr", space="PSUM", bufs=2) as ps_tr,
            tc.tile_pool(name="ps_pp", space="PSUM", bufs=2) as ps_pp,
            tc.tile_pool(name="ps_e", space="PSUM", bufs=2) as ps_e,
            tc.tile_pool(name="ps_sc", space="PSUM", bufs=2) as ps_sc,
        ):
            natps = [nat0p, nat1p, nat2p, nat3p]
            # ---- constants / setup
            ident = cp.tile([128, 128], F32, name="ident")
            make_identity(nc, ident[:])
            identb = cp.tile([128, 128], BF16, name="identb")
            nc.vector.tensor_copy(identb[:], ident[:])
            w_sb = cp.tile([128, KC, ATTN], BF16, name="w_sb")
            nc.gpsimd.dma_start(out=w_sb[:], in_=we_d.rearrange("(k p) a -> p k a", p=128))
            wd_sb = cp.tile([128, KC, ATTN], BF16, name="wd_sb")
            nc.gpsimd.dma_start(out=wd_sb[:], in_=wd_d.rearrange("(k p) a -> p k a", p=128))
            v_sb = cp.tile([128, MC], BF16, name="v_sb")
            nc.gpsimd.dma_start(out=v_sb[:], in_=v_d.rearrange("(m p) -> p m", p=128))
            dh_sb = cp.tile([8, DEC], BF16, name="dh_sb")
            nc.gpsimd.dma_start(out=dh_sb[:], in_=dec_d)
            mp_sb = cp.tile([GS, NQ, T], U8, name="mp_sb")
            nc.sync.dma_start(out=mp_sb[:], in_=msk_d.rearrange("(q j) t -> j q t", j=GS))

            # vmask tiles: column j holds v chunk m, other columns zero
            vm_t = {}
            for m in range(MC):
                for j in range(GS):
                    t = cp.tile([128, GS], BF16, name=f"vm_{m}_{j}")
                    nc.vector.memset(t[:], 0.0)
                    nc.vector.tensor_copy(t[:, j : j + 1], v_sb[:, m : m + 1])
                    vm_t[(m, j)] = t


            # dec_hidden transposed -> [128dec, KC, 8b]
            dechT = cp.tile([128, KC, 8], BF16, name="dechT")
            for k in range(KC):
                pst = ps_sc.tile([128, 512], F32, name="pst", tag="sc")
                nc.tensor.transpose(
                    pst[:].bitcast(BF16)[:, 0:8],
                    dh_sb[0:8, k * 128 : (k + 1) * 128],
                    identb[0:8, 0:8],
                )
                nc.vector.tensor_copy(dechT[:, k, :], pst[:].bitcast(BF16)[:, 0:8])
            # proj_dec -> [128attn, MC, 8b] (tanh bias, f32)
            pdec = cp.tile([128, MC, 8], F32, name="pdec")
            for m in range(MC):
                psd = ps_sc.tile([128, 512], F32, name="psd", tag="sc")
                for k in range(KC):
                    nc.tensor.matmul(
                        psd[:, 0:8],
                        wd_sb[:, k, m * 128 : (m + 1) * 128],
                        dechT[:, k, :],
                        start=(k == 0),
                        stop=(k == KC - 1),
                    )
                nc.scalar.copy(pdec[:, m, :], psd[:, 0:8])

            # ---- main loop over batch pairs
            def emit_context(pend):
                pnats, pwT32, pq = pend
                for j in range(GS):
                    b = GS * pq + j
                    psc = ps_sc.tile([128, 512], F32, name="psc", tag="sc")
                    for ci in range(4):
                        for i in range(4):
                            nc.tensor.matmul(
                                psc[32 * i : 32 * i + 32, :],
                                pwT32[:, j, 4 * i + ci, :],
                                pnats[j][:, 4 * i + ci, :],
                                start=(ci == 0),
                                stop=(ci == 3),
                                tile_position=(0, 32 * i),
                            )
                    cs = smallp.tile([128, 512], F32, name="cs", tag="cs")
                    nc.vector.tensor_copy(cs[:], psc[:])
                    nc.sync.dma_start(
                        out=ctx_d[b],
                        in_=cs[:].rearrange("(i r) e -> i r e", r=32)[:, 0, :],
                    )

            def emit_jg(q, g, j, nats, eps):
                b = GS * q + j
                # transpose enc block [tok,enc]->[enc,tok], 2 k-chunks/tile
                encTs = []
                for kk in range(2):
                    trp = ps_tr.tile([128, 1024], BF16, name="trp", tag="tr")
                    for k2 in range(2):
                        for s in range(4):
                            nc.tensor.transpose(
                                trp[:, k2 * 512 + s * 128 : k2 * 512 + (s + 1) * 128],
                                nats[j][:, 4 * g + s, (2 * kk + k2) * 128 : (2 * kk + k2 + 1) * 128],
                                identb[:],
                            )
                    eT = encTp.tile([128, 1024], BF16, name="eT", tag="encT")
                    nc.vector.tensor_copy(eT[:], trp[:])
                    encTs.append(eT)
                # proj + tanh + energy per attn chunk
                for m in range(MC):
                    pp = ps_pp.tile([128, 512], F32, name="pp", tag="pp")
                    for k in range(KC):
                        nc.tensor.matmul(
                            pp[:],
                            w_sb[:, k, m * 128 : (m + 1) * 128],
                            encTs[k // 2][:, (k % 2) * 512 : (k % 2 + 1) * 512],
                            start=(k == 0),
                            stop=(k == KC - 1),
                        )
                    th = thp.tile([128, 512], BF16, name="th", tag="th")
                    nc.scalar.activation(
                        th[:], pp[:], AF.Tanh, bias=pdec[:, m, b : b + 1], scale=1.0
                    )
                    nc.tensor.matmul(
                        eps[:],
                        vm_t[(m, j)][:],
                        th[:],
                        start=(j == 0 and m == 0),
                        stop=(j == GS - 1 and m == MC - 1),
                    )

            pending = None
            for _rep in range(reps):
              for q in range(NQ):
                nats = []
                for j in range(GS):
                    b = GS * q + j
                    pool = natps[j]
                    nat = pool.tile([128, NCH, 512], BF16, name=f"nat{j}", tag=f"nat{j}")
                    for g in range(NG):
                        nc.gpsimd.dma_start(
                            out=nat[:, 4 * g : 4 * g + 4, :],
                            in_=enc_d[b, g * 512 : (g + 1) * 512, :].rearrange(
                                "(c p) e -> p c e", p=128
                            ),
                        )
                    nats.append(nat)

                e_q = erowp.tile([GS, T], F32, name="e_q", tag="e_q")
                for g in range(NG):
                    eps = ps_e.tile([GS, 512], F32, name="eps", tag="eps")
                    for j in range(GS):
                        emit_jg(q, g, j, nats, eps)
                    # fold mask (-1e30 * m) + energy psum -> sbuf row
                    nc.vector.scalar_tensor_tensor(
                        out=e_q[:, g * 512 : (g + 1) * 512],
                        in0=mp_sb[:, q, g * 512 : (g + 1) * 512],
                        scalar=NEG,
                        in1=eps[:],
                        op0=ALU.mult,
                        op1=ALU.add,
                    )

                # ---- softmax for the pair (no max shift: |e| <= sum|v| ~ 8)
                p_q = prowp.tile([GS, T], F32, name="p_q", tag="p_q")
                den = smallp.tile([GS, 1], F32, name="den", tag="den")
                nc.scalar.activation(
                    p_q[:], e_q[:], AF.Exp, bias=0.0, scale=1.0, accum_out=den[:, 0:1]
                )
                dsafe = smallp.tile([GS, 1], F32, name="dsafe", tag="dsafe")
                nc.vector.tensor_scalar_max(dsafe[:], den[:], 1e-30)
                rec = smallp.tile([GS, 1], F32, name="rec", tag="rec")
                nc.vector.reciprocal(rec[:], dsafe[:])
                nc.vector.tensor_scalar_mul(p_q[:], p_q[:], rec[:, 0:1])
                nc.sync.dma_start(out=att_d[GS * q : GS * q + GS, :], in_=p_q[:])

                # ---- transpose weights into wT32[:, j, c, 0]
                trw = ps_sc.tile([128, 512], F32, name="trw", tag="sc")
                for c in range(NCH):
                    nc.tensor.transpose(
                        trw[:, c * GS : (c + 1) * GS],
                        p_q[:, c * 128 : (c + 1) * 128],
                        ident[0:GS, 0:GS],
                    )
                wT32 = wTp.tile([128, GS, NCH, 32], BF16, name="wT32", tag="wT32")
                nc.vector.memset(wT32[:], 0.0)
                for j in range(GS):
                    nc.vector.tensor_copy(
                        wT32[:, j, :, 0:1],
                        trw[:, j : GS * NCH : GS].rearrange("p (c one) -> p c one", one=1),
                    )
                emit_context((nats, wT32, q))

    nc.compile()
    return nc


_NC = None


def _get_nc():
    global _NC
    if _NC is None:
        _NC = build_program()
    return _NC


def kernel(enc_out, dec_hidden, mask, W_enc, W_dec, v, trace=False, **run_kwargs):
    nc = _get_nc()
    enc_out = np.ascontiguousarray(enc_out, dtype=np.float32)
    dec_hidden = np.ascontiguousarray(dec_hidden, dtype=np.float32)
    mask_u8 = np.ascontiguousarray(mask).astype(np.uint8)
    W_enc = np.ascontiguousarray(W_enc, dtype=np.float32)
    W_dec = np.ascontiguousarray(W_dec, dtype=np.float32)
    v = np.ascontiguousarray(v, dtype=np.float32)

    in_maps = []
    for i in range(NCORES):
        sl = slice(i * BL, (i + 1) * BL)
        in_maps.append(
            {
                "enc_d": enc_out[sl],
                "dec_d": dec_hidden[sl],
                "msk_d": mask_u8[sl],
                "we_d": W_enc,
                "wd_d": W_dec,
                "v_d": v,
            }
        )
    res = run_bass_kernel_spmd(
        nc, in_maps, list(range(NCORES)), trace=trace, **run_kwargs
    )
    ctx = np.concatenate(
        [res.results[i]["ctx_d"].sum(axis=1) for i in range(NCORES)], axis=0
    )
    att = np.concatenate([res.results[i]["att_d"] for i in range(NCORES)], axis=0)
    if trace:
        kernel.last_results = res
    return ctx, att


if __name__ == "__main__":
    rng = np.random.default_rng(0)
    enc = rng.standard_normal((B, T, ENC), dtype=np.float32)
    dec = rng.standard_normal((B, DEC), dtype=np.float32)
    msk = rng.integers(0, 2, size=(B, T)).astype(bool)
    we = rng.uniform(-1, 1, size=(ENC, ATTN)).astype(np.float32) / np.sqrt(ENC)
    wd = rng.uniform(-1, 1, size=(DEC, ATTN)).astype(np.float32) / np.sqrt(DEC)
    vv = rng.uniform(-1, 1, size=(ATTN,)).astype(np.float32) / np.sqrt(ATTN)
    ctx, att = kernel(enc, dec, msk, we, wd, vv)
    print("shapes:", ctx.shape, att.shape)


# revision 13
# speedup vs baseline: 1.3221x; 1.3221x over previous
"""Bahdanau attention Trainium2 kernel (v3: bf16 datapath).

Problem: B=64, T=2048, ENC=512, DEC=512, ATTN=256, fp32 in/out.
  proj_enc = enc_out @ W_enc                         [B,T,A]
  energy   = tanh(proj_enc + dec_hidden@W_dec) . v   [B,T]
  attn_w   = softmax(mask(energy))                   [B,T]
  context  = attn_w @ enc_out                        [B,E]

Sharding: data-parallel over batch across 8 cores (8 batches/core),
weights replicated.  Per core, batches are processed in pairs so the
softmax runs on [2, T] tiles (engine ops need 32-aligned base
partitions) while the pair's enc tiles stay resident in SBUF for the
context matmul (single HBM pass).

enc is cast fp32->bf16 during the DMA load (line rate).  All matmuls
run in bf16 with fp32 PSUM accumulation; softmax runs in fp32.  enc
tiles are PE-transposed (bf16, 1 cyc/row) for the projection matmul.
The context matmul uses 4-way column tiling (tile_position) with the
4 partial sums reduced on the host during unshard.
"""

import sys

sys.path.insert(0, "/opt/trn_rl_repo")

import numpy as np
import concourse.bass as bass
import concourse.tile as tile
from concourse import bacc, mybir
from concourse.bass_utils import run_bass_kernel_spmd
from concourse.masks import make_identity

dt = mybir.dt
F32 = dt.float32
BF16 = dt.bfloat16
U8 = dt.uint8
AF = mybir.ActivationFunctionType
ALU = mybir.AluOpType

B, T, ENC, DEC, ATTN = 64, 2048, 512, 512, 256
NCORES = 8
BL = B // NCORES  # 8 batches per core
GS = 4  # batches per group
NQ = BL // GS  # 2 quads
KC = ENC // 128  # 4 contraction chunks
MC = ATTN // 128  # 2 attn chunks
NG = T // 512  # 4 token groups of 512
NCH = T // 128  # 16 token chunks of 128
NEG = -1.0e30


def build_program(reps=1, timing_mode=False):
    nc = bacc.Bacc("TRN2", target_bir_lowering=False, debug=False)

    enc_kind = "Internal" if timing_mode else "ExternalInput"
    enc_d = nc.dram_tensor("enc_d", [BL, T, ENC], F32, kind=enc_kind).ap()
    dec_d = nc.dram_tensor("dec_d", [BL, DEC], F32, kind="ExternalInput").ap()
    msk_d = nc.dram_tensor("msk_d", [BL, T], U8, kind="ExternalInput").ap()
    we_d = nc.dram_tensor("we_d", [ENC, ATTN], F32, kind="ExternalInput").ap()
    wd_d = nc.dram_tensor("wd_d", [DEC, ATTN], F32, kind="ExternalInput").ap()
    v_d = nc.dram_tensor("v_d", [ATTN], F32, kind="ExternalInput").ap()
    ctx_d = nc.dram_tensor("ctx_d", [BL, 4, ENC], F32, kind="ExternalOutput").ap()
    att_d = nc.dram_tensor("att_d", [BL, T], F32, kind="ExternalOutput").ap()

    with tile.TileContext(nc) as tc:
        with (
            tc.tile_pool(name="consts", bufs=1) as cp,
            tc.tile_pool(name="nat0p", bufs=2) as nat0p,
            tc.tile_pool(name="nat1p", bufs=2) as nat1p,
            tc.tile_pool(name="nat2p", bufs=2) as nat2p,
            tc.tile_pool(name="nat3p", bufs=2) as nat3p,
            tc.tile_pool(name="encTp", bufs=6) as encTp,
            tc.tile_pool(name="thp", bufs=6) as thp,
            tc.tile_pool(name="erow", bufs=2) as erowp,
            tc.tile_pool(name="prow", bufs=2) as prowp,
            tc.tile_pool(name="smallp", bufs=4) as smallp,
            tc.tile_pool(name="wTp", bufs=2) as wTp,
            tc.tile_pool(name="ps_tr", space="PSUM", bufs=2) as ps_tr,
            tc.tile_pool(name="ps_pp", space="PSUM", bufs=2) as ps_pp,
            tc.tile_pool(name="ps_e", space="PSUM", bufs=2) as ps_e,
            tc.tile_pool(name="ps_sc", space="PSUM", bufs=2) as ps_sc,
        ):
            natps = [nat0p, nat1p, nat2p, nat3p]
            # ---- constants / setup
            ident = cp.tile([128, 128], F32, name="ident")
            make_identity(nc, ident[:])
            identb = cp.tile([128, 128], BF16, name="identb")
            nc.vector.tensor_copy(identb[:], ident[:])
            w_sb = cp.tile([128, KC, ATTN], BF16, name="w_sb")
            nc.gpsimd.dma_start(out=w_sb[:], in_=we_d.rearrange("(k p) a -> p k a", p=128))
            wd_sb = cp.tile([128, KC, ATTN], BF16, name="wd_sb")
            nc.gpsimd.dma_start(out=wd_sb[:], in_=wd_d.rearrange("(k p) a -> p k a", p=128))
            v_sb = cp.tile([128, MC], BF16, name="v_sb")
            nc.gpsimd.dma_start(out=v_sb[:], in_=v_d.rearrange("(m p) -> p m", p=128))
            dh_sb = cp.tile([8, DEC], BF16, name="dh_sb")
            nc.gpsimd.dma_start(out=dh_sb[:], in_=dec_d)
            mp_sb = cp.tile([GS, NQ, T], U8, name="mp_sb")
            nc.sync.dma_start(out=mp_sb[:], in_=msk_d.rearrange("(q j) t -> j q t", j=GS))

            # vmask tiles: column j holds v chunk m, other columns zero
            vm_t = {}
            for m in range(MC):
                for j in range(GS):
                    t = cp.tile([128, GS], BF16, name=f"vm_{m}_{j}")
                    nc.vector.memset(t[:], 0.0)
                    nc.vector.tensor_copy(t[:, j : j + 1], v_sb[:, m : m + 1])
                    vm_t[(m, j)] = t


            # dec_hidden transposed -> [128dec, KC, 8b]
            dechT = cp.tile([128, KC, 8], BF16, name="dechT")
            for k in range(KC):
                pst = ps_sc.tile([128, 512], F32, name="pst", tag="sc")
                nc.tensor.transpose(
                    pst[:].bitcast(BF16)[:, 0:8],
                    dh_sb[0:8, k * 128 : (k + 1) * 128],
                    identb[0:8, 0:8],
                )
                nc.vector.tensor_copy(dechT[:, k, :], pst[:].bitcast(BF16)[:, 0:8])
            # proj_dec -> [128attn, MC, 8b] (tanh bias, f32)
            pdec = cp.tile([128, MC, 8], F32, name="pdec")
            for m in range(MC):
                psd = ps_sc.tile([128, 512], F32, name="psd", tag="sc")
                for k in range(KC):
                    nc.tensor.matmul(
                        psd[:, 0:8],
                        wd_sb[:, k, m * 128 : (m + 1) * 128],
                        dechT[:, k, :],
                        start=(k == 0),
                        stop=(k == KC - 1),
                    )
                nc.scalar.copy(pdec[:, m, :], psd[:, 0:8])

            # ---- main loop over batch pairs
            def emit_context(pend):
                pnats, pwT32, pq = pend
                for j in range(GS):
                    b = GS * pq + j
                    psc = ps_sc.tile([128, 512], F32, name="psc", tag="sc")
                    for ci in range(4):
                        for i in range(4):
                            nc.tensor.matmul(
                                psc[32 * i : 32 * i + 32, :],
                                pwT32[:, j, 4 * i + ci, :],
                                pnats[j][:, 4 * i + ci, :],
                                start=(ci == 0),
                                stop=(ci == 3),
                                tile_position=(0, 32 * i),
                            )
                    cs = smallp.tile([128, 512], F32, name="cs", tag="cs")
                    nc.vector.tensor_copy(cs[:], psc[:])
                    nc.sync.dma_start(
                        out=ctx_d[b],
                        in_=cs[:].rearrange("(i r) e -> i r e", r=32)[:, 0, :],
                    )

            def emit_jg(q, g, j, nats, eps):
                b = GS * q + j
                # transpose enc block [tok,enc]->[enc,tok], 2 k-chunks/tile
                encTs = []
                for kk in range(2):
                    trp = ps_tr.tile([128, 1024], BF16, name="trp", tag="tr")
                    for k2 in range(2):
                        for s in range(4):
                            nc.tensor.transpose(
                                trp[:, k2 * 512 + s * 128 : k2 * 512 + (s + 1) * 128],
                                nats[j][:, 4 * g + s, (2 * kk + k2) * 128 : (2 * kk + k2 + 1) * 128],
                                identb[:],
                            )
                    eT = encTp.tile([128, 1024], BF16, name="eT", tag="encT")
                    nc.vector.tensor_copy(eT[:], trp[:])
                    encTs.append(eT)
                # proj + tanh + energy per attn chunk
                for m in range(MC):
                    pp = ps_pp.tile([128, 512], F32, name="pp", tag="pp")
                    for k in range(KC):
                        nc.tensor.matmul(
                            pp[:],
                            w_sb[:, k, m * 128 : (m + 1) * 128],
                            encTs[k // 2][:, (k % 2) * 512 : (k % 2 + 1) * 512],
                            start=(k == 0),
                            stop=(k == KC - 1),
                        )
                    th = thp.tile([128, 512], BF16, name="th", tag="th")
                    nc.scalar.activation(
                        th[:], pp[:], AF.Tanh, bias=pdec[:, m, b : b + 1], scale=1.0
                    )
                    nc.tensor.matmul(
                        eps[:],
                        vm_t[(m, j)][:],
                        th[:],
                        start=(j == 0 and m == 0),
                        stop=(j == GS - 1 and m == MC - 1),
                    )

            pending = None
            for _rep in range(reps):
              for q in range(NQ):
                nats = []
                for j in range(GS):
                    nat = natps[j].tile(
                        [128, NCH, 512], BF16, name=f"nat{j}", tag=f"nat{j}"
                    )
                    nats.append(nat)
                for g in range(NG):
                    for j in range(GS):
                        b = GS * q + j
                        nc.gpsimd.dma_start(
                            out=nats[j][:, 4 * g : 4 * g + 4, :],
                            in_=enc_d[b, g * 512 : (g + 1) * 512, :].rearrange(
                                "(c p) e -> p c e", p=128
                            ),
                        )

                e_q = erowp.tile([GS, T], F32, name="e_q", tag="e_q")
                for g in range(NG):
                    eps = ps_e.tile([GS, 512], F32, name="eps", tag="eps")
                    for j in range(GS):
                        emit_jg(q, g, j, nats, eps)
                    # fold mask (-1e30 * m) + energy psum -> sbuf row
                    nc.vector.scalar_tensor_tensor(
                        out=e_q[:, g * 512 : (g + 1) * 512],
                        in0=mp_sb[:, q, g * 512 : (g + 1) * 512],
                        scalar=NEG,
                        in1=eps[:],
                        op0=ALU.mult,
                        op1=ALU.add,
                    )

                # ---- softmax for the pair (no max shift: |e| <= sum|v| ~ 8)
                p_q = prowp.tile([GS, T], F32, name="p_q", tag="p_q")
                den = smallp.tile([GS, 1], F32, name="den", tag="den")
                nc.scalar.activation(
                    p_q[:], e_q[:], AF.Exp, bias=0.0, scale=1.0, accum_out=den[:, 0:1]
                )
                dsafe = smallp.tile([GS, 1], F32, name="dsafe", tag="dsafe")
                nc.vector.tensor_scalar_max(dsafe[:], den[:], 1e-30)
                rec = smallp.tile([GS, 1], F32, name="rec", tag="rec")
                nc.vector.reciprocal(rec[:], dsafe[:])
                nc.vector.tensor_scalar_mul(p_q[:], p_q[:], rec[:, 0:1])
                nc.sync.dma_start(out=att_d[GS * q : GS * q + GS, :], in_=p_q[:])

                # ---- transpose weights into wT32[:, j, c, 0]
                trw = ps_sc.tile([128, 512], F32, name="trw", tag="sc")
                for c in range(NCH):
                    nc.tensor.transpose(
                        trw[:, c * GS : (c + 1) * GS],
                        p_q[:, c * 128 : (c + 1) * 128],
                        ident[0:GS, 0:GS],
                    )
                wT32 = wTp.tile([128, GS, NCH, 32], BF16, name="wT32", tag="wT32")
                nc.vector.memset(wT32[:], 0.0)
                for j in range(GS):
                    nc.vector.tensor_copy(
                        wT32[:, j, :, 0:1],
                        trw[:, j : GS * NCH : GS].rearrange("p (c one) -> p c one", one=1),
                    )
                emit_context((nats, wT32, q))

    nc.compile()
    return nc


_NC = None


def _get_nc():
    global _NC
    if _NC is None:
        _NC = build_program()
    return _NC


def kernel(enc_out, dec_hidden, mask, W_enc, W_dec, v, trace=False, **run_kwargs):
    nc = _get_nc()
    enc_out = np.ascontiguousarray(enc_out, dtype=np.float32)
    dec_hidden = np.ascontiguousarray(dec_hidden, dtype=np.float32)
    mask_u8 = np.ascontiguousarray(mask).astype(np.uint8)
    W_enc = np.ascontiguousarray(W_enc, dtype=np.float32)
    W_dec = np.ascontiguousarray(W_dec, dtype=np.float32)
    v = np.ascontiguousarray(v, dtype=np.float32)

    in_maps = []
    for i in range(NCORES):
        sl = slice(i * BL, (i + 1) * BL)
        in_maps.append(
            {
                "enc_d": enc_out[sl],
                "dec_d": dec_hidden[sl],
                "msk_d": mask_u8[sl],
                "we_d": W_enc,
                "wd_d": W_dec,
                "v_d": v,
            }
        )
    res = run_bass_kernel_spmd(
        nc, in_maps, list(range(NCORES)), trace=trace, **run_kwargs
    )
    ctx = np.concatenate(
        [res.results[i]["ctx_d"].sum(axis=1) for i in range(NCORES)], axis=0
    )
    att = np.concatenate([res.results[i]["att_d"] for i in range(NCORES)], axis=0)
    if trace:
        kernel.last_results = res
    return ctx, att


if __name__ == "__main__":
    rng = np.random.default_rng(0)
    enc = rng.standard_normal((B, T, ENC), dtype=np.float32)
    dec = rng.standard_normal((B, DEC), dtype=np.float32)
    msk = rng.integers(0, 2, size=(B, T)).astype(bool)
    we = rng.uniform(-1, 1, size=(ENC, ATTN)).astype(np.float32) / np.sqrt(ENC)
    wd = rng.uniform(-1, 1, size=(DEC, ATTN)).astype(np.float32) / np.sqrt(DEC)
    vv = rng.uniform(-1, 1, size=(ATTN,)).astype(np.float32) / np.sqrt(ATTN)
    ctx, att = kernel(enc, dec, msk, we, wd, vv)
    print("shapes:", ctx.shape, att.shape)
